# revision 3
# baseline (speedup 1.0000x reference)
"""RNN-T Joiner kernel for 8 Trainium2 NeuronCores.

out[b,t,u,:] = tanh(enc[b,t,:] + pred[b,u,:]) @ W.T + b

Sharding: 2 cores per batch, each takes half the t range (200 t), so every
core owns one batch and 20000 (t,u) cells. Data path is bf16 end-to-end
(enc/pred/W/logits/output) with f32 PSUM accumulation; the 2e-2 rel-err
budget dwarfs the ~0.5% bf16 error.

Per core engine split (budgeted against the PE's ~137us of matmuls):
  - producers (logit = enc[c,t] + pred[c,u], broadcast add): DVE runs at 1x
    on broadcast APs, so units are spread over DVE / GpSimd / fused-ACT
    (per-t tanh with per-partition bias) by a static table.
  - ACT: big in-place tanh per (ck, block)
  - PE: psum[cells, v] += logit[c, cells].T @ W[c, v], 4 chunk matmuls per
    128-cell tile, 4 tiles (banks) per psum group
  - DVE: one bias-add per 4-bank group (psum f32 + bias f32 -> bf16 sbuf)
  - DMA: 512KB bf16 stores per group

Constants (W, f32 bias via bitcast, enc slice, pred) are packed host-side
into one bf16 [128, NCOL] tensor -> a single input DMA.
"""

import sys

sys.path.insert(0, "/opt/trn_rl_repo")

import numpy as np
import ml_dtypes

import concourse.bass as bass
import concourse.bacc as bacc
import concourse.mybir as mybir
from concourse.tile import TileContext
from concourse.bass_utils import run_bass_kernel_spmd

B, T, U, C, V = 4, 400, 100, 512, 512
NCORES = 8
TSC = T // 2  # 200 t per core (2 cores per batch)
P = 128
CK = C // P  # 4 contraction chunks
CELLS = TSC * U  # 20000 cells per core
F32 = mybir.dt.float32
BF16 = mybir.dt.bfloat16
BF = ml_dtypes.bfloat16

# t-blocks: 6 blocks of 32 t (3200 cells = 25 tiles) + 1 block of 8 t
# (800 cells) => 157 matmul tiles of <=128 cells, no tile straddles blocks.
BLOCK_T = [32] * 6 + [8]
BLOCK_CELLS = [bt * U for bt in BLOCK_T]
BLOCK_C0 = np.cumsum([0] + BLOCK_CELLS).tolist()  # cell offset per block
NBLK = len(BLOCK_T)

# mm tiles: (cell_start, m)
TILES = [(s, P) for s in range(0, (CELLS // P) * P, P)]
if CELLS % P:
    TILES.append(((CELLS // P) * P, CELLS % P))
# psum groups of up to 4 tiles (4 banks)
GROUPS = [TILES[i : i + 4] for i in range(0, len(TILES), 4)]

# packed consts layout (bf16 columns)
W_OFF = 0  # [ck, v] -> 4*512
BIAS_OFF = W_OFF + CK * V  # f32 bias replicated [4, 512], stored as 2x bf16
ENC_OFF = BIAS_OFF + 2 * 4 * V  # [ck, t] -> 4*200
PRED_OFF = ENC_OFF + CK * TSC  # [ck, u] -> 4*100
NCOL = PRED_OFF + CK * U  # 7344

# producer unit engine assignment: unit = blk*4 + ck for full blocks (0..23),
# small block 6 units are 24..27. Balanced for PE-bound (~140us) operation:
# DVE also carries ~89us of bias-add consumers.
GPS_UNITS = {1, 3, 5, 7, 9, 11, 13, 15, 17, 19, 21, 23}  # GpSimd adds
ACTF_UNITS = {2, 10, 18}  # fused per-t add+tanh on ACT

_cache = {}


def _build():
    nc = bacc.Bacc("TRN2", target_bir_lowering=False, debug=False)
    consts = nc.declare_dram_parameter("consts", [P, NCOL], BF16, isOutput=False)
    out = nc.declare_dram_parameter("out", [TSC, U, V], BF16, isOutput=True)
    ob = out.ap().rearrange("t u v -> (t u) v")  # [20000, 512]

    with TileContext(nc) as tc:
        with (
            tc.tile_pool(name="consts", bufs=1) as cpool,
            tc.tile_pool(name="logit", bufs=2) as logit_pool,
            tc.tile_pool(name="osb", bufs=4) as out_pool,
            tc.tile_pool(name="psum", bufs=2, space="PSUM") as psum_pool,
        ):
            cs = cpool.tile([P, NCOL], BF16, tag="cs")
            nc.sync.dma_start(out=cs, in_=consts.ap())

            wview = cs[:, W_OFF : W_OFF + CK * V].rearrange(
                "p (ck v) -> p ck v", ck=CK
            )
            bias_f32 = cs[:, BIAS_OFF : BIAS_OFF + 2 * 4 * V].bitcast(F32)
            eview = cs[:, ENC_OFF : ENC_OFF + CK * TSC].rearrange(
                "p (ck t) -> p ck t", ck=CK
            )
            pview = cs[:, PRED_OFF : PRED_OFF + CK * U].rearrange(
                "p (ck u) -> p ck u", ck=CK
            )

            lg = {}  # (blk, ck) -> flat [P, cells] bf16 view

            def emit_producer(blk, ck):
                bt = BLOCK_T[blk]
                t0 = sum(BLOCK_T[:blk])
                ncell = bt * U
                lgt = logit_pool.tile([P, 3200], BF16, tag=f"lg{ck}")
                lg[(blk, ck)] = lgt
                v3 = lgt[:, :ncell].rearrange("p (t u) -> p t u", t=bt)
                unit = blk * 4 + ck
                if unit in ACTF_UNITS:
                    # fused add+tanh, one ACT op per t (bias is per-partition)
                    for t in range(bt):
                        nc.scalar.activation(
                            out=v3[:, t, :],
                            in_=pview[:, ck, :],
                            func=mybir.ActivationFunctionType.Tanh,
                            bias=eview[:, ck, t0 + t : t0 + t + 1],
                        )
                else:
                    e_col = (
                        eview[:, ck, t0 : t0 + bt]
                        .unsqueeze(2)
                        .broadcast_to([P, bt, U])
                    )
                    p_row = (
                        pview[:, ck, :].unsqueeze(1).broadcast_to([P, bt, U])
                    )
                    eng = nc.gpsimd if unit in GPS_UNITS else nc.vector
                    eng.tensor_add(out=v3, in0=e_col, in1=p_row)
                    nc.scalar.activation(
                        out=lgt[:, :ncell],
                        in_=lgt[:, :ncell],
                        func=mybir.ActivationFunctionType.Tanh,
                    )

            def emit_group(g):
                tiles = GROUPS[g]
                ps = psum_pool.tile([P, 4 * V], F32, tag="ps")
                for j, (s, m) in enumerate(tiles):
                    blk = min(s // 3200, NBLK - 1)
                    off = s - BLOCK_C0[blk]
                    for ck in range(CK):
                        nc.tensor.matmul(
                            ps[:m, j * V : (j + 1) * V],
                            lhsT=lg[(blk, ck)][:, off : off + m],
                            rhs=wview[:, ck, :],
                            start=(ck == 0),
                            stop=(ck == CK - 1),
                        )
                ncol = len(tiles) * V
                mlast = tiles[-1][1]
                osb = out_pool.tile([P, 4 * V], BF16, tag="osb")
                nc.vector.tensor_add(
                    out=osb[:, :ncol], in0=ps[:, :ncol], in1=bias_f32[:, :ncol]
                )
                # full 128-cell tiles in one strided DMA; ragged tail alone
                nfull = len(tiles) - (1 if mlast != P else 0)
                c0 = tiles[0][0]
                if nfull:
                    dst = ob[c0 : c0 + nfull * P, :].rearrange(
                        "(j p) v -> p j v", p=P
                    )
                    src = osb[:, : nfull * V].rearrange(
                        "p (j v) -> p j v", v=V
                    )
                    nc.sync.dma_start(out=dst, in_=src)
                if mlast != P:
                    s, m = tiles[-1]
                    nc.sync.dma_start(
                        out=ob[s : s + m, :],
                        in_=osb[:m, (len(tiles) - 1) * V : ncol],
                    )

            # interleave producers and consumers so Tile pipelines blocks
            next_g = 0
            for blk in range(NBLK):
                for ck in range(CK):
                    emit_producer(blk, ck)
                done = BLOCK_C0[blk + 1]
                while next_g < len(GROUPS) and (
                    GROUPS[next_g][-1][0] + GROUPS[next_g][-1][1] <= done
                ):
                    emit_group(next_g)
                    next_g += 1
            while next_g < len(GROUPS):
                emit_group(next_g)
                next_g += 1
    nc.compile()
    return nc


def _install_ntff_hook():
    """This image's antenv lacks axon_hooks; wire the ctypes NTFF hook from
    trn_boot against the axon PJRT .so so trace=True works."""
    if "antenv.axon_hooks" in sys.modules:
        return
    import types

    holder = [None]
    mod = types.ModuleType("antenv.axon_hooks")
    mod.set_axon_ntff_profile_hook = lambda h: holder.__setitem__(0, h)
    mod.get_axon_ntff_profile_hook = lambda: holder[0]
    sys.modules["antenv.axon_hooks"] = mod
    try:
        sys.path.insert(0, "/root/.axon_site/trn_agent_boot")
        from trn_boot import _ntff_profile_via_ctypes

        mod.set_axon_ntff_profile_hook(
            _ntff_profile_via_ctypes("/opt/axon/libaxon_pjrt.so")
        )
    except Exception as e:  # degrade to no tracing
        print(f"NTFF hook install failed: {e}", file=sys.stderr)


def _run(in_maps, trace=False, tmpdir=None):
    if "nc" not in _cache:
        _cache["nc"] = _build()
    if trace:
        _install_ntff_hook()
    return run_bass_kernel_spmd(
        _cache["nc"], in_maps, list(range(NCORES)), trace=trace, tmpdir=tmpdir
    )


def make_in_maps(encoder_out, predictor_out, W, b):
    encoder_out = np.asarray(encoder_out, dtype=np.float32)
    predictor_out = np.asarray(predictor_out, dtype=np.float32)
    W = np.asarray(W, dtype=np.float32)
    b = np.asarray(b, dtype=np.float32)

    # [p, ck, v] <- W[v, ck*P+p]
    w_pack = W.reshape(V, CK, P).transpose(2, 1, 0).reshape(P, CK * V)
    bias_rep = np.tile(b, (P, 4, 1)).reshape(P, 4 * V).astype(np.float32)
    bias_bf = bias_rep.view(BF)  # [P, 2*4*V] raw f32 bytes as bf16 cols

    in_maps = []
    for i in range(NCORES):
        bb, half = i // 2, i % 2
        base = np.zeros((P, NCOL), BF)
        base[:, W_OFF : W_OFF + CK * V] = w_pack.astype(BF)
        base[:, BIAS_OFF : BIAS_OFF + 2 * 4 * V] = bias_bf
        enc_s = encoder_out[bb, half * TSC : (half + 1) * TSC, :]  # [t, c]
        base[:, ENC_OFF : ENC_OFF + CK * TSC] = (
            enc_s.reshape(TSC, CK, P).transpose(2, 1, 0).reshape(P, -1)
        ).astype(BF)
        base[:, PRED_OFF : PRED_OFF + CK * U] = (
            predictor_out[bb].reshape(U, CK, P).transpose(2, 1, 0).reshape(P, -1)
        ).astype(BF)
        in_maps.append({"consts": base})
    return in_maps


def gather(results):
    full = np.empty((B, T, U, V), np.float32)
    for i in range(NCORES):
        bb, half = i // 2, i % 2
        full[bb, half * TSC : (half + 1) * TSC] = np.asarray(
            results[i]["out"]
        ).astype(np.float32)
    return full


def kernel(encoder_out, predictor_out, W, b):
    in_maps = make_in_maps(encoder_out, predictor_out, W, b)
    res = _run(in_maps, trace=False)
    return gather(res.results)
